# revision 54
# baseline (speedup 1.0000x reference)
"""EnhancedChannelFilter Trainium2 kernel (packed-pixel bf16 pipeline).

Full inputs in, full outputs out. Internally: pure data-parallel over 8
NeuronCores (4 images each), NCHW layout with channels on SBUF partitions.

Structural facts exploited (all host-derivable from shapes + the scalar
Packet_Loss_Rate input + the reference's fixed RNG seed; none depend on x):
  * The packet-loss mask zeroes 1472-byte chunks == runs of 23 units in
    (pixel, 16-channel-group) raster space. At rate=50, ~34% of pixels have
    ALL 16 groups dead -> x_m column is exactly 0 -> det col 0 ->
    z col = [0, sigmoid(0)*0] = 0 -> rec1/rec2 cols 0 -> output col 0.
    Those pixels are compacted away on the host (selection only); the device
    computes on packed columns tiled in <=448-pixel n-tiles. Images are dealt
    to cores by descending kept-pixel count so all cores share one program
    whose per-slot tail tile shrinks to that slot's max.
  * adapt = rate*adapt_w - threshold <= -1 makes relu(sigmoid(.)+adapt)
    exactly 0 for every image (sigmoid < 1), killing ~42% of OUTPUT
    channels: rec2 rows, the final scale, and the out DMA are packed to the
    alive channel set only (157 of 256 here). Host scatters the packed bf16
    result into the f32 zeros output.
  * Group-level masking of kept pixels still runs on device: the host ships
    a pre-expanded per-channel 0/1 mask as fp8 (exact) and the DVE applies
    it to the bf16 x in the same scalar_tensor_tensor that accumulates the
    SE row-sums (f32 accum_out), so masking costs no extra pass.

Per core, per image:
  1. x [2,128,P] bf16 + mex [128,2,P] fp8 DMA'd in halves for early start;
     x_m = x * mask fused with the SE row-sum on DVE.
  2. SE chain (fc1 -> relu -> fc2 -> sigmoid -> +bias -> relu) on PE/ACT in
     alive-channel-packed space; 1/HW and rate*adapt_w - threshold folded
     into the weights on the host; dedicated 1-bank PSUM pool.
  3. det / rec1 / rec2 1x1-conv GEMMs in bf16 (1 row/cycle, FWL weight
     loads), K on partitions, PSUM f32, one 1-bank PSUM tile per (j, m-half)
     with bufs>=2 so PE double-buffers against the ACT evictions.
  4. sigmoid/relu PSUM evictions on ACT (bf16 out), zh = sigmoid(det)*x_m
     on Pool, final per-alive-channel scale on DVE (tensor_scalar from
     PSUM), paired n-tiles per out DMA, bf16 out.
"""

import math

import numpy as np
import ml_dtypes

B, C, H, W = 32, 256, 56, 56
HW = H * W              # 3136
NCORES = 8
BC = B // NCORES        # images per core
NT = 448                # pixels per n-tile
EPC = 1472 // 4         # f32 elements per packet chunk (368)
QG = 16                 # channel-group size: gcd(EPC, C)
UPC = EPC // QG         # 23 channel-group-units per chunk

_CACHE: dict = {}


# ---------------------------------------------------------------------------
# Workaround: this walrus build enforces 1 sync wait per instruction (2 for
# EventSemaphore), but the Tile framework attaches several to its exit drain.
# Splitting extra waits onto dedicated same-engine NOPs placed immediately
# before the instruction is semantically identical.
# ---------------------------------------------------------------------------
def _split_multiwaits(nc, mybir):
    n = 0
    for bb in nc.m.functions[0].blocks:
        lst = bb.instructions
        for inst in list(lst):
            si = inst.sync_info
            if si is None or not si.on_wait:
                continue
            cap = 2 if isinstance(inst, mybir.InstEventSemaphore) else 1
            waits = list(si.on_wait)
            if len(waits) <= cap:
                continue
            eng = nc.engines[inst.engine]
            extra = []
            for wt in waits[:-cap]:
                nop = eng.nop(nofuse=True).ins
                nop.sync_info = mybir.SyncInfo(on_wait=[wt], on_update=[])
                nc.cur_bb.bb.instructions.remove(nop)
                extra.append(nop)
            si.on_wait = waits[-cap:]
            idx = lst.index(inst)
            lst[idx:idx] = extra
            n += 1
    return n


def _build(repeat=0, debug=False, pipeline=1, psum=(3, 2, 2), sb=(2, 3, 3, 3, 3), og=2, odve=0):
    import concourse.bass as bass
    import concourse.tile as tile
    import concourse.mybir as mybir

    lay = _CACHE["layout"]
    TW = lay["tilew"]                    # per-slot n-tile width lists
    P = lay["pmax"]                      # DRAM row length (max over slots)
    NA = lay["na"]                       # alive output channels
    MTW = [128, NA - 128] if NA > 128 else [NA]   # m-tile widths for rec2/out

    f32 = mybir.dt.float32
    bf16 = mybir.dt.bfloat16
    fp8 = mybir.dt.float8e4
    DR = mybir.MatmulPerfMode.DoubleRow
    MULT = mybir.AluOpType.mult
    BYPASS = mybir.AluOpType.bypass
    SIGMOID = mybir.ActivationFunctionType.Sigmoid
    COPY = mybir.ActivationFunctionType.Copy
    RELU = mybir.ActivationFunctionType.Relu

    nc = bass.Bass("TRN2", target_bir_lowering=False, debug=False)

    x_d = nc.dram_tensor("x", [BC, 2, 128, P], bf16, kind="ExternalInput").ap()
    # pre-expanded per-channel 0/1 mask, [img, channel(128), half, pixel]
    mex_d = nc.dram_tensor("mex", [BC, 128, 2, P], fp8, kind="ExternalInput").ap()
    # bf16 GEMM weights packed column-wise: det (k2 x m2) [128,128], rec1
    # (k4 x m2) [128,128], rec2 (k2 x MTW) packed alive rows.
    NWG = 12 * 128 + 2 * sum(MTW)
    wgemm_d = nc.dram_tensor("wgemm", [128, NWG], bf16, kind="ExternalInput").ap()
    # [16, .]-partition smalls: wfc2 packed-alive [16, NA] f32
    wsm_d = nc.dram_tensor("wsm", [16, NA], f32, kind="ExternalInput").ap()
    # [128, .] f32 smalls: fc1 2x[128,16], abias per m-tile [., 1]
    wf32_d = nc.dram_tensor("wf32", [128, 32 + len(MTW)], f32,
                            kind="ExternalInput").ap()
    out_d = nc.dram_tensor("out", [BC, NA, P], bf16, kind="ExternalOutput").ap()
    if debug:
        dxm_d = nc.dram_tensor("dxm", [BC, 2, 128, P], f32, kind="ExternalOutput").ap()
        dmc_d = nc.dram_tensor("dmc", [BC, 128, len(MTW)], f32,
                               kind="ExternalOutput").ap()
        dy_d = nc.dram_tensor("dy", [BC, 2, 128, 8], f32, kind="ExternalOutput").ap()

    with tile.TileContext(nc) as tc:
        with (
            tc.tile_pool(name="consts", bufs=1) as cpool,
            tc.tile_pool(name="xin", bufs=sb[0]) as xpool,
            tc.tile_pool(name="xm", bufs=sb[1]) as xmpool,
            tc.tile_pool(name="m16", bufs=sb[5] if len(sb) > 5 else 2) as m16pool,
            tc.tile_pool(name="sg", bufs=sb[2]) as sgpool,
            tc.tile_pool(name="zh", bufs=sb[3]) as zhpool,
            tc.tile_pool(name="r1", bufs=sb[4]) as r1pool,
            tc.tile_pool(name="osb", bufs=sb[6] if len(sb) > 6 else 2) as opool,
            tc.tile_pool(name="ysum", bufs=2) as ypool,
            tc.tile_pool(name="mch", bufs=4) as mcpool,
            tc.tile_pool(name="sesb", bufs=2) as sepool,
            tc.tile_pool(name="sep", bufs=1, space="PSUM") as seppool,
            tc.tile_pool(name="dp", bufs=psum[0], space="PSUM") as dppool,
            tc.tile_pool(name="r1p", bufs=psum[1], space="PSUM") as r1ppool,
            tc.tile_pool(name="r2p", bufs=psum[2], space="PSUM") as r2ppool,
        ):
            # ---- constants into SBUF (4 DMAs) ----
            wgemm = cpool.tile([128, NWG], bf16, name="wgemm", tag="wgemm")
            wsm = cpool.tile([16, NA], f32, name="wsm", tag="wsm")
            wf32 = cpool.tile([128, 32 + len(MTW)], f32, name="wf32", tag="wf32")
            if repeat:
                nc.sync.dma_start(wgemm[:], wgemm_d[:])

            wofs = [0]
            for wdt in [128] * 12 + list(MTW) * 2:
                wofs.append(wofs[-1] + wdt)

            def _wcol(i):
                return wgemm[:, wofs[i]:wofs[i + 1]]

            wdet_sb = [[_wcol(k * 2 + m) for m in range(2)] for k in range(2)]
            wrec1_sb = [[_wcol(4 + k * 2 + m) for m in range(2)] for k in range(4)]
            wrec2_sb = [[_wcol(12 + k * len(MTW) + m) for m in range(len(MTW))]
                        for k in range(2)]
            wfc1_sb = [wf32[:, k * 16:(k + 1) * 16] for k in range(2)]
            abias_sb = [wf32[:, 32 + m:33 + m] for m in range(len(MTW))]
            aofs = [0]
            for wdt in MTW:
                aofs.append(aofs[-1] + wdt)
            wfc2_sb = [wsm[:, aofs[m]:aofs[m + 1]] for m in range(len(MTW))]

            st = {}
            m16st = {}

            def phase1(b):
                W = TW[b]
                PB = sum(W)
                nj = len(W)
                cx = (PB // 2) // 64 * 64
                mex_sb = m16pool.tile([128, 2, PB], fp8, name=f"mex_b{b}", tag="m16")
                nc.sync.dma_start(mex_sb[:, :, 0:cx], mex_d[b, :, :, 0:cx])
                x_sb = [xpool.tile([128, PB], bf16, name=f"x_b{b}h{h}", tag=f"x{h}")
                        for h in range(2)]
                for h in range(2):
                    nc.sync.dma_start(x_sb[h][:, 0:cx], x_d[b, h, :, 0:cx])
                if b == 0 and not repeat:
                    # det tiles first so the first GEMM isn't gated on the
                    # full weight blob
                    nc.sync.dma_start(wgemm[:, 0:512], wgemm_d[:, 0:512])
                    nc.sync.dma_start(wgemm[:, 512:NWG], wgemm_d[:, 512:NWG])
                nc.sync.dma_start(mex_sb[:, :, cx:PB], mex_d[b, :, :, cx:PB])
                for h in range(2):
                    nc.sync.dma_start(x_sb[h][:, cx:PB], x_d[b, h, :, cx:PB])
                if b == 0:
                    nc.sync.dma_start(wsm[:], wsm_d[:])
                    nc.sync.dma_start(wf32[:], wf32_d[:])

                xm_sb = [xmpool.tile([128, PB], bf16, name=f"xm_b{b}h{h}", tag=f"xm{h}")
                         for h in range(2)]
                ysum = [ypool.tile([128, 8], f32, name=f"ysum_b{b}h{h}", tag=f"ysum{h}")
                        for h in range(2)]

                n0 = 0
                for j, wj in enumerate(W):
                    for h in range(2):
                        nc.vector.scalar_tensor_tensor(
                            out=xm_sb[h][:, n0:n0 + wj],
                            in0=x_sb[h][:, n0:n0 + wj],
                            scalar=0.0,
                            in1=mex_sb[:, h, n0:n0 + wj],
                            op0=BYPASS,
                            op1=MULT,
                            accum_out=ysum[h][:, j:j + 1],
                        )
                    n0 += wj
                if debug:
                    for h in range(2):
                        nc.sync.dma_start(dxm_d[b, h, :, 0:PB], xm_sb[h][:])

                # SE chain -> per-alive-channel output scale mc[m]
                for h in range(2):
                    nc.vector.reduce_sum(ysum[h][:, 7:8], ysum[h][:, 0:nj],
                                         axis=mybir.AxisListType.X)
                fc1p = seppool.tile([16, 1], f32, name=f"fc1p_b{b}", tag="sep")
                nc.tensor.matmul(fc1p[:], wfc1_sb[0][:], ysum[0][:, 7:8],
                                 start=True, stop=False)
                nc.tensor.matmul(fc1p[:], wfc1_sb[1][:], ysum[1][:, 7:8],
                                 start=False, stop=True)
                h1 = sepool.tile([16, 1], f32, name=f"h1_b{b}", tag="h1")
                nc.scalar.activation(h1[:], fc1p[:], RELU)
                mc = []
                for m, mw in enumerate(MTW):
                    scp = seppool.tile([128, 1], f32, name=f"scp_b{b}m{m}", tag="sep")
                    nc.tensor.matmul(scp[0:mw], wfc2_sb[m][:], h1[:],
                                     start=True, stop=True)
                    ssb = sepool.tile([128, 1], f32, name=f"ssb_b{b}m{m}", tag="ssb")
                    nc.scalar.activation(ssb[0:mw], scp[0:mw], SIGMOID)
                    mch = mcpool.tile([128, 1], f32, name=f"mc_b{b}m{m}", tag="mc")
                    nc.scalar.activation(mch[0:mw], ssb[0:mw], RELU,
                                         bias=abias_sb[m][0:mw])
                    mc.append(mch)
                if debug:
                    for h in range(2):
                        nc.sync.dma_start(dy_d[b, h], ysum[h][:])
                    for m in range(len(MTW)):
                        nc.sync.dma_start(dmc_d[b, :, m:m + 1], mc[m][:])
                st[b] = (xm_sb, mc)

            def phase2(b):
                W = TW[b]
                xm_sb, mc = st[b]
                ot = {}
                n0 = 0
                for j, wj in enumerate(W):
                    xmn = [xm_sb[h][:, n0:n0 + wj] for h in range(2)]

                    sg = sgpool.tile([128, 2 * NT], bf16, name=f"sg_b{b}j{j}", tag="sg")
                    for m in range(2):
                        dp = dppool.tile([128, 512], f32, name=f"dp_b{b}j{j}m{m}",
                                         tag="dp")
                        for k in range(2):
                            nc.tensor.matmul(
                                dp[:, 0:wj],
                                wdet_sb[k][m][:], xmn[k],
                                start=(k == 0), stop=(k == 1),
                            )
                        nc.scalar.activation(
                            sg[:, m * NT:m * NT + wj], dp[:, 0:wj], SIGMOID)
                    zh = []
                    for h in range(2):
                        z = zhpool.tile([128, NT], bf16, name=f"zh_b{b}h{h}j{j}",
                                        tag=f"zh{h}")
                        nc.gpsimd.tensor_tensor(
                            z[0:128, 0:wj], sg[:, h * NT:h * NT + wj], xmn[h], MULT,
                        )
                        zh.append(z)

                    kts = [xmn[0], xmn[1], zh[0][0:128, 0:wj], zh[1][0:128, 0:wj]]
                    r1sb = r1pool.tile([128, 2 * NT], bf16, name=f"r1_b{b}j{j}", tag="r1")
                    for m in range(2):
                        r1p = r1ppool.tile([128, 512], f32, name=f"r1p_b{b}j{j}m{m}",
                                           tag="r1p")
                        for k in range(4):
                            nc.tensor.matmul(
                                r1p[:, 0:wj],
                                wrec1_sb[k][m][:], kts[k],
                                start=(k == 0), stop=(k == 3),
                            )
                        nc.scalar.activation(
                            r1sb[:, m * NT:m * NT + wj], r1p[:, 0:wj], RELU)

                    for m, mw in enumerate(MTW):
                        r2p = r2ppool.tile([128, NT], f32, name=f"r2p_b{b}m{m}j{j}",
                                           tag="r2p")
                        for k in range(2):
                            nc.tensor.matmul(
                                r2p[0:mw, 0:wj],
                                wrec2_sb[k][m][:],
                                r1sb[:, k * NT:k * NT + wj],
                                start=(k == 0), stop=(k == 1),
                            )
                        # group og n-tiles into one [mw, <=og*NT] out tile
                        # per m: one DMA per (m, n-group) instead of per (m, n)
                        if j % og == 0:
                            ot[m] = opool.tile([128, og * NT], bf16,
                                               name=f"o_b{b}m{m}j{j}", tag=f"o{m}")
                        o = ot[m][0:mw, (j % og) * NT:(j % og) * NT + wj]
                        nc.vector.tensor_scalar_mul(o, r2p[0:mw, 0:wj],
                                                    mc[m][0:mw])
                        if j % og == og - 1 or j == len(W) - 1:
                            # non-last tiles are always full NT wide, so the
                            # group is contiguous in the tile: cols [0, pw)
                            pw = (j % og) * NT + wj
                            pn0 = n0 - (j % og) * NT
                            oeng = nc.gpsimd if odve else nc.sync
                            oeng.dma_start(
                                out_d[b, aofs[m]:aofs[m + 1], pn0:pn0 + pw],
                                ot[m][0:mw, 0:pw])
                    n0 += wj

            import contextlib as _ctxlib
            rep_cm = (tc.For_i(0, repeat, 1,
                               hint_engines=(mybir.EngineType.PE,
                                             mybir.EngineType.DVE,
                                             mybir.EngineType.Activation,
                                             mybir.EngineType.SP,
                                             mybir.EngineType.Pool))
                      if repeat else _ctxlib.nullcontext())
            with rep_cm:
                if pipeline == 2:
                    phase1(0)
                    phase1(1)
                    for b in range(BC):
                        phase2(b)
                        if b + 2 < BC:
                            phase1(b + 2)
                        del st[b]
                elif pipeline:
                    phase1(0)
                    for b in range(BC):
                        if b + 1 < BC:
                            phase1(b + 1)
                        phase2(b)
                        del st[b]
                else:
                    for b in range(BC):
                        phase1(b)
                        phase2(b)
                        del st[b]

    _split_multiwaits(nc, mybir)
    return nc


def _jax_perm_cpu(num_chunks: int) -> np.ndarray:
    """jax.random.permutation(key(1234), num_chunks) on the CPU backend.

    Run in a JAX_PLATFORMS=cpu subprocess: in this process jax may be bound
    to an accelerator backend that cannot lower the shuffle's sort op.
    """
    import os
    import subprocess
    import sys
    import tempfile

    import jax

    sp = os.path.dirname(os.path.dirname(jax.__file__))
    code = (
        "import sys, numpy as np, jax\n"
        f"perm = np.asarray(jax.random.permutation(jax.random.key(1234), {num_chunks}))\n"
        "np.save(sys.argv[1], perm)\n"
    )
    with tempfile.TemporaryDirectory() as td:
        path = os.path.join(td, "perm.npy")
        env = dict(os.environ, JAX_PLATFORMS="cpu", PYTHONPATH=sp)
        env.pop("TRN_TERMINAL_POOL_IPS", None)
        subprocess.run([sys.executable, "-c", code, path], env=env, check=True)
        return np.load(path)


def _group_mask(rate: int) -> np.ndarray:
    """Per-image [HW, 16] f32 keep-mask in (pixel, channel-group) space."""
    n = B * C * HW
    num_chunks = math.ceil(n * 4 / 1472)
    num_lossy = int(math.ceil(num_chunks * (rate / 100)))
    keep = np.ones((num_chunks,), np.float32)
    if num_lossy > 0:
        perm = _jax_perm_cpu(num_chunks)
        keep[perm[:num_lossy]] = 0.0
    bg = np.arange(B, dtype=np.int64)
    qq = np.arange(QG, dtype=np.int64)
    pp = np.arange(HW, dtype=np.int64)
    u = (bg[:, None, None] * HW + pp[:, None]) * QG + qq[None, None, :]
    return keep[u // UPC]          # [B, HW, 16]


def _prep_in_maps(inputs):
    x = np.asarray(inputs["x"], dtype=np.float32)
    rate = int(np.asarray(inputs["Packet_Loss_Rate"]))
    fc1 = np.asarray(inputs["fc1_w"], dtype=np.float32)
    fc2 = np.asarray(inputs["fc2_w"], dtype=np.float32)
    thr = float(np.asarray(inputs["threshold"], dtype=np.float32).reshape(-1)[0])
    detw = np.asarray(inputs["detect_w"], dtype=np.float32)
    r1w = np.asarray(inputs["rec1_w"], dtype=np.float32)
    r2w = np.asarray(inputs["rec2_w"], dtype=np.float32)
    aw = np.asarray(inputs["adapt_w"], dtype=np.float32)

    km = _group_mask(rate)                     # [B, HW, 16]
    pixel_alive = km.any(axis=2)               # [B, HW]
    keep_idx = [np.nonzero(pixel_alive[b])[0] for b in range(B)]
    kcnt = np.array([len(k) for k in keep_idx])
    kmax = int(kcnt.max(initial=0))
    ab = (rate * aw[:, 0] - thr).astype(np.float32)
    alive_idx = np.nonzero(ab > -1.0)[0]
    na = len(alive_idx)
    if kmax == 0 or na == 0:
        _CACHE["layout"] = {"tilew": [], "pmax": 0, "na": na,
                            "keep_idx": keep_idx, "alive_idx": alive_idx,
                            "img_of": None}
        return None
    # Deal images to cores by descending kept-pixel count so every core's
    # slot s shares one tile-width list; the tail tile shrinks to fit the
    # slot's max K (rounded to 64).
    seq = np.argsort(-kcnt, kind="stable")
    img_of = seq.reshape(BC, NCORES)           # [slot, core] -> image
    tilew = []
    for s in range(BC):
        mk = int(kcnt[img_of[s]].max())
        ntiles = max(1, math.ceil(mk / NT))
        tail = mk - (ntiles - 1) * NT
        tailw = min(NT, math.ceil(tail / 16) * 16)
        tilew.append([NT] * (ntiles - 1) + [tailw])
    pmax = max(sum(w) for w in tilew)
    MTW = [128, na - 128] if na > 128 else [na]
    _CACHE["layout"] = {"tilew": tilew, "pmax": pmax, "na": na,
                        "keep_idx": keep_idx, "alive_idx": alive_idx,
                        "img_of": img_of}
    P = pmax

    # x packed: [B, 2, 128, P] bf16 ; mask expanded: [B, 128, 2, P] fp8
    # (indexed [slot*NCORES + core] after dealing)
    xr = x.reshape(B, C, HW)
    xp = np.zeros((B, C, P), ml_dtypes.bfloat16)
    mex = np.zeros((B, 256, P), ml_dtypes.float8_e4m3)
    for b in range(B):
        g = int(seq[b])
        ki = keep_idx[g]
        xp[b, :, 0:len(ki)] = xr[g][:, ki]
        mex[b, :, 0:len(ki)] = np.repeat(km[g][ki].T, QG, axis=0)
    xp = xp.reshape(B, 2, 128, P)
    # [img, h, c, p] -> [img, c, h, p]
    mex = np.ascontiguousarray(mex.reshape(B, 2, 128, P).transpose(0, 2, 1, 3))

    # bf16 GEMM weights packed column-wise (see _build's wofs layout)
    detT, r1T, r2T = detw.T, r1w.T, r2w.T
    r2Tp = r2T[:, alive_idx]                   # [256, na]
    cols = []
    for k in range(2):
        for m in range(2):
            cols.append(detT[k * 128:(k + 1) * 128, m * 128:(m + 1) * 128])
    for k in range(4):
        for m in range(2):
            cols.append(r1T[k * 128:(k + 1) * 128, m * 128:(m + 1) * 128])
    for k in range(2):
        o = 0
        for mw in MTW:
            cols.append(r2Tp[k * 128:(k + 1) * 128, o:o + mw])
            o += mw
    wgemm = np.concatenate(cols, axis=1).astype(ml_dtypes.bfloat16)

    wsm = np.ascontiguousarray(fc2.T[:, alive_idx].astype(np.float32))  # [16, na]
    wf32 = np.zeros((128, 32 + len(MTW)), np.float32)
    fc1T = fc1.T / HW                                  # [256, 16]
    wf32[:, 0:16] = fc1T[0:128]
    wf32[:, 16:32] = fc1T[128:256]
    abp = ab[alive_idx]
    o = 0
    for m, mw in enumerate(MTW):
        wf32[0:mw, 32 + m] = abp[o:o + mw]
        o += mw

    in_maps = []
    for c in range(NCORES):
        in_maps.append({
            "x": np.ascontiguousarray(xp[c::NCORES]),
            "mex": np.ascontiguousarray(mex[c::NCORES]),
            "wgemm": wgemm, "wsm": wsm, "wf32": wf32,
        })
    return in_maps


def _unpack_out(res) -> np.ndarray:
    lay = _CACHE["layout"]
    keep_idx, alive_idx = lay["keep_idx"], lay["alive_idx"]
    img_of = lay["img_of"]
    out_full = np.zeros((B, C, HW), np.float32)
    for c in range(NCORES):
        oc = np.asarray(res.results[c]["out"])
        for s in range(BC):
            g = int(img_of[s][c])
            ki = keep_idx[g]
            out_full[g][np.ix_(alive_idx, ki)] = oc[s][:, 0:len(ki)].astype(
                np.float32)
    return out_full.reshape(B, C, H, W)


def kernel(**inputs) -> np.ndarray:
    from concourse.bass_utils import run_bass_kernel_spmd

    in_maps = _prep_in_maps(inputs)
    if in_maps is None:        # fully-lossy or no alive channel: output is 0
        return np.zeros((B, C, H, W), np.float32)
    lay = _CACHE["layout"]
    key = ("nc", tuple(tuple(w) for w in lay["tilew"]), lay["na"])
    if key not in _CACHE:
        _CACHE[key] = _build()
    nc = _CACHE[key]
    last_err = None
    for _attempt in range(4):
        try:
            res = run_bass_kernel_spmd(nc, in_maps, core_ids=list(range(NCORES)))
            break
        except Exception as e:  # transient axon/device hiccups: retry
            last_err = e
    else:
        raise last_err
    return _unpack_out(res)
